# revision 46
# baseline (speedup 1.0000x reference)
"""Trainium2 Bass kernel for the DeepEquilibriumModel (Anderson-accelerated DEQ).

Problem: 12 unrolled iterations of
    f(z) = tanh(z @ W1 + x @ Wx + b1) @ W2 + b2
with Anderson mixing (M=5, beta=1, lam=1e-4) from iteration 5 on.

Sharding: pure data parallelism over the 2048 = B*S rows; 8 cores get 256
rows each (cores 0-3 hold batch 0, cores 4-7 batch 1). Weights replicated.
The Anderson normal equations need global row sums per batch element, done
with a tiny per-group AllReduce (groups {0..3} / {4..7}).

v3 structure (tuned to measured engine rates):
  * h-loop: per f-chunk, ps1 = identity@xwx + sum_k W1[k,f]@z[k]; tanh to a
    full h tile.  W2 phase is m-outer (one PSUM bank at a time) so f, g and
    the Gram dot partials trail each m-chunk instead of waiting for all.
  * dots are m-chunked (16 small DVE ops + 4 ACT squares) -> only ~2.5us of
    tail after the last W2 matmul before the AllReduce can launch.
  * Early iterations (0..3) do NO AllReduce: the Gram matrix P is built from
    local partials and reduced ONCE at i=4 (P is linear in the dots), which
    avoids backing up the collective stream.
  * z_{i+1} = sum_k c_k f_{i-k} runs on the PE as 5 scaled-identity matmuls
    per k-chunk (identC_k built by DVE from the broadcast coefficients),
    with ACT copying PSUM->SBUF; ~3us instead of ~8us of DVE stt chain.
  * 4x4 solve via vectorized Gauss-Jordan (SPD + lam*I, no pivoting) on
    [1,4,5] views; builds are fused (~26 tiny DVE ops total).
  * HAM keep-warm: free-running dummy matmuls fill the AllReduce wait and
    solve-keyed dummies tick the PE through the Gauss-Jordan so the clock
    gate never drops the PE to 1.2 GHz mid-iteration.
"""

import numpy as np

from concourse import bacc, bass, mybir, tile
from concourse.bass_utils import run_bass_kernel_spmd

import os as _os

B, S, D, F = 2, 1024, 512, 2048
MAX_ITER, M, LAM = int(_os.environ.get("K_ITERS", "12")), 5, 1e-4
NCORES = 8
RPC = (B * S) // NCORES      # rows per core = 256
KD = D // 128                # 4 k-chunks over D
KF = F // 128                # 16 k-chunks over F
MD = D // 128                # 4 output chunks over D
NDUM = int(_os.environ.get("K_NDUM", "0"))
NCH = int(_os.environ.get("K_NCH", "0"))

FP = mybir.dt.float32
FPR = mybir.dt.float32r
BF = mybir.dt.bfloat16
ALU = mybir.AluOpType
ACT = mybir.ActivationFunctionType

RGROUPS = [[0, 1, 2, 3], [4, 5, 6, 7]]
WT = BF


def _f32(ap):
    return ap.bitcast(FP)


def _emit(nc: bass.Bass):
    v = nc.vector
    sc = nc.scalar
    gp = nc.gpsimd

    # ---------------- DRAM I/O ----------------
    xT_d = nc.dram_tensor("xT", [D, RPC], WT, kind="ExternalInput")
    W1_d = nc.dram_tensor("W1", [D, F], WT, kind="ExternalInput")
    Wx_d = nc.dram_tensor("Wx", [D, F], WT, kind="ExternalInput")
    W2_d = nc.dram_tensor("W2", [F, D], WT, kind="ExternalInput")
    b1_d = nc.dram_tensor("b1", [F], FP, kind="ExternalInput")
    b2_d = nc.dram_tensor("b2", [D], FP, kind="ExternalInput")
    zout_d = nc.dram_tensor("zT_out", [D, RPC], FP, kind="ExternalOutput")

    with tile.TileContext(nc) as tc:
        with (
            tc.tile_pool(name="const", bufs=1) as cp,
            tc.tile_pool(name="hbp", bufs=3) as hp,
            tc.tile_pool(name="ps1p", bufs=2, space="PSUM") as pp1,
            tc.tile_pool(name="ps2p", bufs=2, space="PSUM") as pp2,
            tc.tile_pool(name="pszp", bufs=1, space="PSUM") as ppz,
            tc.tile_pool(name="pssm", bufs=1, space="PSUM") as pps,
            tc.tile_pool(name="dram", bufs=2, space="DRAM") as dp,
        ):
            # ---------------- constants / weights ----------------
            W1p = cp.tile([128, KD * F], WT)          # (k,f) at [:, k*F + f*128]
            W2p = cp.tile([128, KF * D], WT)          # (f,m) at [:, f*D + m*128]
            xwxp = cp.tile([128, KF * RPC], WT)       # f at [:, f*RPC]
            b1t = cp.tile([128, KF], FP)
            b2t = cp.tile([128, MD], FP)
            ones_col = cp.tile([128, 1], FP)
            ones_row = cp.tile([1, 128], FP)
            onesq = cp.tile([128, 128], FP)
            identR = cp.tile([128, 128], WT)
            ident5 = cp.tile([128, M * 128], WT)
            identAll = cp.tile([128, M * 128], WT)

            nc.sync.dma_start(b1t[:], b1_d.ap().rearrange("(f p) -> p f", p=128))
            nc.sync.dma_start(b2t[:], b2_d.ap().rearrange("(m p) -> p m", p=128))
            v.memset(ones_col[:], 1.0)
            v.memset(ones_row[:], 1.0)
            v.memset(onesq[:], 1.0)
            gp.affine_select(onesq[:], onesq[:], [[1, 128]], ALU.is_equal, 0.0,
                             base=0, channel_multiplier=-1)
            v.tensor_copy(identR[:], onesq[:])
            v.tensor_copy(ident5[:].rearrange("p (j c) -> p j c", j=M),
                          onesq[:].rearrange("p (j c) -> p j c", j=1)
                               .broadcast_to([128, M, 128]))

            # -------- state tiles --------
            fh = [cp.tile([128, KD * RPC], WT, name=f"fh{j}") for j in range(M)]
            gh = [cp.tile([128, KD * RPC], BF, name=f"gh{j}") for j in range(M)]
            z0 = cp.tile([128, KD * RPC], WT)
            z1 = cp.tile([128, KD * RPC], WT)
            z320 = cp.tile([128, KD * RPC], FP)
            z321 = cp.tile([128, KD * RPC], FP)
            hfull = cp.tile([128, KF * RPC], WT)
            junkV = cp.tile([128, RPC], BF)
            junkA = cp.tile([128, RPC], BF)
            dAm = cp.tile([128, 4], FP)               # <g,g> partials by m
            dVm = cp.tile([128, 16], FP)              # (j,m) partials, j-major
            redp = cp.tile([1, 24], FP)
            red4 = cp.tile([1, 196], FP)
            t98 = cp.tile([1, 98], FP)
            red2 = cp.tile([1, 49], FP)
            r5 = cp.tile([1, 5], FP)
            Pg = [cp.tile([1, 25], FP, name=f"pg{j}") for j in range(2)]
            Au = cp.tile([1, 20], FP)      # augmented [HTH | HTy] as [1,4,5]
            u4 = cp.tile([1, 4], FP)
            st4 = cp.tile([1, 4], FP)
            rcp = cp.tile([1, 1], FP)
            rowp = cp.tile([1, 5], FP)
            t45 = cp.tile([1, 20], FP)
            csum = cp.tile([1, 1], FP)
            coeffs = cp.tile([1, 5], FP)
            dumout = cp.tile([1, 4], FP)
            pacev = cp.tile([1, 4], WT)
            chain = [cp.tile([1, 64], WT, name=f"ch{j}") for j in range(NCH + 1)]

            v.memset(dAm[:], 0.0)
            v.memset(dVm[:], 0.0)
            v.memset(redp[:], 0.0)
            v.memset(Pg[0][:], 0.0)
            v.memset(Pg[1][:], 0.0)

            # warm up the collective path (first AllReduce after load pays a
            # large one-time latency).
            n_warm = int(_os.environ.get("K_CC_WARMUP", "2"))

            def warm_ar():
                wcc_in = dp.tile([1, 49], FP, tag="cci", name="wcci")
                wcc_ag = dp.tile([4, 49], FP, tag="cco", name="wccag")
                gp.dma_start(wcc_in[0:1, 0:24], redp[:])
                gp.collective_compute(
                    "AllGather", ALU.bypass, replica_groups=RGROUPS,
                    ins=[wcc_in.opt()], outs=[wcc_ag.opt()],
                )

            for w in range(n_warm):
                warm_ar()

            # ---------------- prolog: xwx = Wx.T @ xT + b1 ----------------
            with tc.tile_pool(name="prolog", bufs=1) as pro:
                xTs = pro.tile([128, KD * RPC], WT)
                Wxp = pro.tile([128, KD * F], WT)
                # two hardware DMA queues in parallel: Wx on the scalar
                # engine's queue, everything else on sync
                for k in range(KD):
                    sc.dma_start(Wxp[:, k * F:(k + 1) * F],
                                 Wx_d[k * 128:(k + 1) * 128, :])
                for k in range(KD):
                    nc.sync.dma_start(xTs[:, k * RPC:(k + 1) * RPC],
                                      xT_d[k * 128:(k + 1) * 128, :])
                for f in range(KF):
                    nc.sync.dma_start(W2p[:, f * D:(f + 1) * D],
                                      W2_d[f * 128:(f + 1) * 128, :])
                for k in range(KD):
                    nc.sync.dma_start(W1p[:, k * F:(k + 1) * F],
                                      W1_d[k * 128:(k + 1) * 128, :])
                for f in range(KF):
                    ps1 = pp1.tile([128, RPC], FP, tag="ps1", name="ps1")
                    for k in range(KD):
                        nc.tensor.matmul(
                            ps1[:],
                            Wxp[:, k * F + f * 128: k * F + (f + 1) * 128],
                            xTs[:, k * RPC:(k + 1) * RPC],
                            start=(k == 0), stop=(k == KD - 1),
                        )
                    sc.activation(xwxp[:, f * RPC:(f + 1) * RPC], ps1[:],
                                  ACT.Identity, bias=b1t[:, f:f + 1], scale=1.0)

            # ---------------- main loop (fully unrolled) ----------------
            z_mm = None   # bf16 AP of z_i for matmuls (None for i=0 -> zeros)
            z_sub = None  # AP used by the g subtraction (fp32 from i=6 on)
            for i in range(MAX_ITER):
                slot = i % M
                last = (i == MAX_ITER - 1)
                f_t, g_t = fh[slot], gh[slot]

                if i == M:
                    warm_ar()  # re-warm the collective path before the
                               # first Anderson gather
                # ---- h phase: hfull = tanh(z @ W1 + xwx) ----
                for f in range(KF):
                    fs = slice(f * RPC, (f + 1) * RPC)
                    if i == 0:
                        sc.activation(hfull[:, fs], xwxp[:, fs], ACT.Tanh)
                        continue
                    ps1 = pp1.tile([128, RPC], FP, tag="ps1", name="ps1")
                    for k in range(KD):
                        nc.tensor.matmul(
                            ps1[:],
                            W1p[:, k * F + f * 128: k * F + (f + 1) * 128],
                            z_mm[:, k * RPC:(k + 1) * RPC],
                            start=(k == 0), stop=(k == KD - 1),
                        )
                    # xwx folded in on the DVE (PSUM read), tanh from SBUF
                    hb = hp.tile([128, RPC], BF, tag="hb", name="hb")
                    v.scalar_tensor_tensor(hb[:], ps1[:], 1.0, xwxp[:, fs],
                                           op0=ALU.bypass, op1=ALU.add)
                    sc.activation(hfull[:, fs], hb[:], ACT.Tanh)

                # ---- W2 phase, m-outer; f/g/dot partials trail each m ----
                njd = min(i, M - 1)
                for m in range(MD):
                    ms = slice(m * RPC, (m + 1) * RPC)
                    ps2 = pp2.tile([128, RPC], FP, tag="ps2", name="ps2")
                    for f in range(KF):
                        nc.tensor.matmul(
                            ps2[:],
                            W2p[:, f * D + m * 128: f * D + (m + 1) * 128],
                            hfull[:, f * RPC:(f + 1) * RPC],
                            start=(f == 0), stop=(f == KF - 1),
                        )
                    sc.activation(f_t[:, ms], ps2[:],
                                  ACT.Identity, bias=b2t[:, m:m + 1], scale=1.0)
                    if i == 0:
                        v.tensor_scalar(g_t[:, ms], ps2[:], b2t[:, m:m + 1],
                                        None, op0=ALU.add)
                    else:
                        v.scalar_tensor_tensor(g_t[:, ms], ps2[:],
                                               b2t[:, m:m + 1], z_sub[:, ms],
                                               op0=ALU.add, op1=ALU.subtract)
                    sc.activation(junkA[:], g_t[:, ms], ACT.Square,
                                  accum_out=dAm[:, m:m + 1])
                    for j in range(1, njd + 1):
                        v.scalar_tensor_tensor(
                            junkV[:], g_t[:, ms], 1.0, gh[(i - j) % M][:, ms],
                            op0=ALU.bypass, op1=ALU.mult,
                            accum_out=dVm[:, (j - 1) * 4 + m:(j - 1) * 4 + m + 1])

                # ---- partition-reduce dot partials ----
                pball = pps.tile([128, 32], FP, tag="psmall", name="pball")
                psd = pball[0:1, 0:20]
                nc.tensor.matmul(psd[:, 0:4], ones_col[:], dAm[:],
                                 start=True, stop=True)
                nc.tensor.matmul(psd[:, 4:20], ones_col[:], dVm[:],
                                 start=True, stop=True)
                do_ar = i >= M
                if do_ar:
                    cc_in = dp.tile([1, 49], FP, tag="cci", name="cci")
                    cc_ag = dp.tile([4, 49], FP, tag="cco", name="ccag")
                    sc.activation(redp[:, 0:20], psd, ACT.Copy)
                    nc.sync.dma_start(cc_in[0:1, 0:20], redp[:, 0:20])
                    if i == M:
                        # fuse the early-phase Gram reduction into the same
                        # collective: ship the local P alongside the dots
                        nc.sync.dma_start(cc_in[0:1, 24:49], Pg[(i + 1) % 2][:])
                    gp.collective_compute(
                        "AllGather", ALU.bypass, replica_groups=RGROUPS,
                        ins=[cc_in.opt()], outs=[cc_ag.opt()],
                    )

                # HAM keep-warm: one long accumulation group of junk matmuls
                # (closed after the solve and READ once, so DCE keeps them)
                # fills the AllReduce wait; solve-keyed members tick the PE
                # through the Gauss-Jordan.
                pdum = None
                if do_ar and (NDUM > 0 or NCH > 0):
                    pdum = pps.tile([1, 512], FP, tag="dum", name="pdum")
                    for k in range(NDUM):
                        nc.tensor.matmul(pdum[:], identR[0:1, 0:1],
                                         xwxp[0:1, 0:512],
                                         start=(k == 0), stop=False)
                    # DMA-chain-paced ticks: each link lands ~1.3us after the
                    # previous, giving the PE a heartbeat through the
                    # collective wait at negligible power.
                    for k in range(NCH):
                        if k == 0:
                            # anchor the chain to this iteration's tail: g_t
                            # is finished exactly when the dots ship out
                            sc.dma_start(chain[1][:], g_t[0:1, 0:64])
                        else:
                            sc.dma_start(chain[k + 1][:], chain[k][:])
                        nc.tensor.matmul(pdum[0:1, 0:64], identR[0:1, 0:1],
                                         chain[k + 1][:],
                                         start=(NDUM == 0 and k == 0),
                                         stop=False)

                if do_ar:
                    nc.sync.dma_start(red4[:],
                                      cc_ag[:].rearrange("a b -> (a b)"))
                    # sum the 4 ranks' partials, then the 4 m-partials
                    v.tensor_tensor(t98[:], red4[:, 0:98], red4[:, 98:196],
                                    op=ALU.add)
                    v.tensor_tensor(red2[:], t98[:, 0:49], t98[:, 49:98],
                                    op=ALU.add)
                    v.tensor_reduce(r5[:],
                                    red2[:, 0:20].rearrange(
                                        "p (j m) -> p j m", j=5),
                                    axis=mybir.AxisListType.X, op=ALU.add)
                else:
                    v.tensor_reduce(r5[:],
                                    psd.rearrange("p (j m) -> p j m", j=5),
                                    axis=mybir.AxisListType.X, op=ALU.add)
                    if i in (2, 4):
                        warm_ar()  # keep the collective path warm

                Pc, Pp = Pg[i % 2], Pg[(i + 1) % 2]
                P3c = Pc[:].rearrange("p (a b) -> p a b", a=5)
                P3p = Pp[:].rearrange("p (a b) -> p a b", a=5)
                if i == M:
                    # previous P arrives globally-reduced in the payload
                    v.tensor_copy(Pp[:], red2[:, 24:49])

                if i < M:
                    # ---- P shift + insert (r5: [<g,g>, j1..j4]) ----
                    v.tensor_copy(P3c[:, 1:5, 1:5], P3p[:, 0:4, 0:4])
                    v.tensor_copy(Pc[:, 0:5], r5[:, 0:5])
                    v.tensor_copy(Pc[:, 5:25:5], r5[:, 1:5])
                    z_mm = f_t[:]
                    z_sub = f_t[:]
                    continue

                # ---- augmented [HTH + lam I | HTy] straight from r5 and the
                # OLD P (the shifted-P copy happens off the critical path) ----
                A3 = Au[:].rearrange("p (a b) -> p a b", a=4)
                # t[a,b] = r5[a] - P_old[a-1,b-1]
                v.tensor_tensor(A3[:, :, 0:4],
                                r5[:, 1:5].rearrange("p (a b) -> p a b", b=1)
                                          .broadcast_to([1, 4, 4]),
                                P3p[:, 0:4, 0:4], op=ALU.subtract)
                # u4[b] = r5[0] - r5[b]  (equals HTy as well)
                v.scalar_tensor_tensor(u4[:], r5[:, 1:5], -1.0,
                                       r5[:, 0:1].broadcast_to([1, 4]),
                                       op0=ALU.mult, op1=ALU.add)
                v.tensor_tensor(A3[:, :, 0:4],
                                u4[:].rearrange("p (a b) -> p a b", a=1)
                                     .broadcast_to([1, 4, 4]),
                                A3[:, :, 0:4], op=ALU.subtract)
                v.tensor_scalar(st4[:], Au[:, 0:19:6], LAM, None, op0=ALU.add)
                v.tensor_copy(Au[:, 0:19:6], st4[:])
                v.tensor_copy(A3[:, :, 4:5],
                              u4[:].rearrange("p (a b) -> p a b", b=1))
                # P shift + insert for the next iteration (not on the path)
                v.tensor_copy(P3c[:, 1:5, 1:5], P3p[:, 0:4, 0:4])
                v.tensor_copy(Pc[:, 0:5], r5[:, 0:5])
                v.tensor_copy(Pc[:, 5:25:5], r5[:, 1:5])

                # ---- Gauss-Jordan (no pivoting; HTH is SPD + lam I) ----
                # a dummy PE matmul after each pivot keeps the clock hot
                for p in range(4):
                    v.reciprocal(rcp[:], Au[:, p * 6:p * 6 + 1])
                    v.tensor_scalar(rowp[:], Au[:, p * 5:(p + 1) * 5], rcp[:],
                                    None, op0=ALU.mult)
                    v.tensor_tensor(t45[:].rearrange("p (a b) -> p a b", a=4),
                                    A3[:, :, p:p + 1].broadcast_to([1, 4, 5]),
                                    rowp[:].rearrange("p (a b) -> p a b", a=1)
                                           .broadcast_to([1, 4, 5]),
                                    op=ALU.mult)
                    v.tensor_tensor(A3, A3,
                                    t45[:].rearrange("p (a b) -> p a b", a=4),
                                    op=ALU.subtract)
                    v.tensor_copy(Au[:, p * 5:(p + 1) * 5], rowp[:])
                    if pdum is not None:
                        # pace the PE through the solve with all-f32r members
                        v.tensor_copy(pacev[:], rowp[:, 0:4])
                        nc.tensor.matmul(pdum[0:1, 0:4], identR[0:1, 0:1],
                                         pacev[:], start=False, stop=False)

                # gamma = Au[:, 4:20:5]; coeffs = [1 - sum(gamma), gamma]
                v.tensor_reduce(csum[:], Au[:, 4:20:5],
                                axis=mybir.AxisListType.X, op=ALU.add)
                v.tensor_scalar(coeffs[:, 0:1], csum[:], -1.0, 1.0,
                                op0=ALU.mult, op1=ALU.add)
                v.tensor_copy(coeffs[:, 1:5], Au[:, 4:20:5])

                # broadcast coeffs to all partitions, build scaled identities
                psb = pball[:, 20:25]
                nc.tensor.matmul(psb, ones_row[:], coeffs[:],
                                 start=True, stop=True)
                v.tensor_tensor(
                    identAll[:].rearrange("p (j c) -> p j c", j=M),
                    ident5[:].rearrange("p (j c) -> p j c", j=M),
                    psb[:].rearrange("p (j c) -> p j c", c=1)
                          .broadcast_to([128, M, 128]),
                    op=ALU.mult)

                # close + read the keep-warm group so it survives DCE
                if pdum is not None:
                    v.tensor_copy(pacev[:], coeffs[:, 0:4])
                    nc.tensor.matmul(pdum[0:1, 0:4], identR[0:1, 0:1],
                                     pacev[:], start=False, stop=True)
                    sc.activation(dumout[:], pdum[0:1, 0:4], ACT.Copy)

                # ---- z_{i+1} = sum_k c_k f_{i-k} on the PE ----
                zn = z0 if (i % 2 == 0) else z1
                zn32 = z320 if (i % 2 == 0) else z321
                psz = ppz.tile([128, KD * RPC], FP, tag="psz", name="psz")
                for half in range(2):
                    hs = slice(half * 2 * RPC, (half + 1) * 2 * RPC)
                    for j in range(M):
                        nc.tensor.matmul(psz[:, hs],
                                         identAll[:, j * 128:(j + 1) * 128],
                                         fh[(i - j) % M][:, hs],
                                         start=(j == 0), stop=(j == M - 1))
                for kc in range(KD):
                    ks = slice(kc * RPC, (kc + 1) * RPC)
                    sc.activation(zn[:, ks], psz[:, ks], ACT.Identity)
                v.tensor_copy(zn32[:], psz[:])
                z_mm = zn[:]
                z_sub = zn32[:]

            for k in range(KD):
                nc.sync.dma_start(zout_d[k * 128:(k + 1) * 128, :],
                                  z_sub[:, k * RPC:(k + 1) * RPC])

    nc.compile()
    nc.finalize()
    return nc


_NC = None


def _get_nc():
    global _NC
    if _NC is None:
        nc = bacc.Bacc(trn_type="TRN2", debug=False, num_devices=NCORES)
        _NC = _emit(nc)
    return _NC


def kernel(**inputs):
    import ml_dtypes
    bf = ml_dtypes.bfloat16
    x = np.ascontiguousarray(np.asarray(inputs["x_input"], dtype=np.float32))
    W1 = np.ascontiguousarray(np.asarray(inputs["W1"], dtype=np.float32).astype(bf))
    Wx = np.ascontiguousarray(np.asarray(inputs["Wx"], dtype=np.float32).astype(bf))
    b1 = np.ascontiguousarray(np.asarray(inputs["b1"], dtype=np.float32))
    W2 = np.ascontiguousarray(np.asarray(inputs["W2"], dtype=np.float32).astype(bf))
    b2 = np.ascontiguousarray(np.asarray(inputs["b2"], dtype=np.float32))

    nc = _get_nc()
    in_maps = []
    for c in range(NCORES):
        b, s0 = c // 4, (c % 4) * RPC
        in_maps.append({
            "xT": np.ascontiguousarray(x[b, s0:s0 + RPC, :].T.astype(bf)),
            "W1": W1, "Wx": Wx, "W2": W2, "b1": b1, "b2": b2,
        })
    res = run_bass_kernel_spmd(nc, in_maps, core_ids=list(range(NCORES)))
    out = np.zeros((B, S, D), np.float32)
    for c, om in enumerate(res.results):
        b, s0 = c // 4, (c % 4) * RPC
        out[b, s0:s0 + RPC, :] = om["zT_out"].T
    return out


# revision 47
# speedup vs baseline: 1.0430x; 1.0430x over previous
"""Trainium2 Bass kernel for the DeepEquilibriumModel (Anderson-accelerated DEQ).

Problem: 12 unrolled iterations of
    f(z) = tanh(z @ W1 + x @ Wx + b1) @ W2 + b2
with Anderson mixing (M=5, beta=1, lam=1e-4) from iteration 5 on.

Sharding: pure data parallelism over the 2048 = B*S rows; 8 cores get 256
rows each (cores 0-3 hold batch 0, cores 4-7 batch 1). Weights replicated.
The Anderson normal equations need global row sums per batch element, done
with a tiny per-group AllReduce (groups {0..3} / {4..7}).

v3 structure (tuned to measured engine rates):
  * h-loop: per f-chunk, ps1 = identity@xwx + sum_k W1[k,f]@z[k]; tanh to a
    full h tile.  W2 phase is m-outer (one PSUM bank at a time) so f, g and
    the Gram dot partials trail each m-chunk instead of waiting for all.
  * dots are m-chunked (16 small DVE ops + 4 ACT squares) -> only ~2.5us of
    tail after the last W2 matmul before the AllReduce can launch.
  * Early iterations (0..3) do NO AllReduce: the Gram matrix P is built from
    local partials and reduced ONCE at i=4 (P is linear in the dots), which
    avoids backing up the collective stream.
  * z_{i+1} = sum_k c_k f_{i-k} runs on the PE as 5 scaled-identity matmuls
    per k-chunk (identC_k built by DVE from the broadcast coefficients),
    with ACT copying PSUM->SBUF; ~3us instead of ~8us of DVE stt chain.
  * 4x4 solve via vectorized Gauss-Jordan (SPD + lam*I, no pivoting) on
    [1,4,5] views; builds are fused (~26 tiny DVE ops total).
  * HAM keep-warm: free-running dummy matmuls fill the AllReduce wait and
    solve-keyed dummies tick the PE through the Gauss-Jordan so the clock
    gate never drops the PE to 1.2 GHz mid-iteration.
"""

import numpy as np

from concourse import bacc, bass, mybir, tile
from concourse.bass_utils import run_bass_kernel_spmd

import os as _os

B, S, D, F = 2, 1024, 512, 2048
MAX_ITER, M, LAM = int(_os.environ.get("K_ITERS", "12")), 5, 1e-4
NCORES = 8
RPC = (B * S) // NCORES      # rows per core = 256
KD = D // 128                # 4 k-chunks over D
KF = F // 128                # 16 k-chunks over F
MD = D // 128                # 4 output chunks over D
NDUM = int(_os.environ.get("K_NDUM", "0"))
NCH = int(_os.environ.get("K_NCH", "0"))

FP = mybir.dt.float32
FPR = mybir.dt.float32r
BF = mybir.dt.bfloat16
ALU = mybir.AluOpType
ACT = mybir.ActivationFunctionType

RGROUPS = [[0, 1, 2, 3], [4, 5, 6, 7]]
WT = BF


def _f32(ap):
    return ap.bitcast(FP)


def _emit(nc: bass.Bass):
    v = nc.vector
    sc = nc.scalar
    gp = nc.gpsimd

    # ---------------- DRAM I/O ----------------
    xT_d = nc.dram_tensor("xT", [D, RPC], WT, kind="ExternalInput")
    W1_d = nc.dram_tensor("W1", [D, F], WT, kind="ExternalInput")
    Wx_d = nc.dram_tensor("Wx", [D, F], WT, kind="ExternalInput")
    W2_d = nc.dram_tensor("W2", [F, D], WT, kind="ExternalInput")
    b1_d = nc.dram_tensor("b1", [F], FP, kind="ExternalInput")
    b2_d = nc.dram_tensor("b2", [D], FP, kind="ExternalInput")
    zout_d = nc.dram_tensor("zT_out", [D, RPC], FP, kind="ExternalOutput")

    with tile.TileContext(nc) as tc:
        with (
            tc.tile_pool(name="const", bufs=1) as cp,
            tc.tile_pool(name="hbp", bufs=3) as hp,
            tc.tile_pool(name="ps1p", bufs=2, space="PSUM") as pp1,
            tc.tile_pool(name="ps2p", bufs=2, space="PSUM") as pp2,
            tc.tile_pool(name="pszp", bufs=2, space="PSUM") as ppz,
            tc.tile_pool(name="pssm", bufs=1, space="PSUM") as pps,
            tc.tile_pool(name="dram", bufs=2, space="DRAM") as dp,
        ):
            # ---------------- constants / weights ----------------
            W1p = cp.tile([128, KD * F], WT)          # (k,f) at [:, k*F + f*128]
            W2p = cp.tile([128, KF * D], WT)          # (f,m) at [:, f*D + m*128]
            xwxp = cp.tile([128, KF * RPC], WT)       # f at [:, f*RPC]
            b1t = cp.tile([128, KF], FP)
            b2t = cp.tile([128, MD], FP)
            ones_col = cp.tile([128, 1], FP)
            ones_row = cp.tile([1, 128], FP)
            onesq = cp.tile([128, 128], FP)
            identR = cp.tile([128, 128], WT)
            ident5 = cp.tile([128, M * 128], WT)
            identAll = cp.tile([128, M * 128], WT)

            nc.sync.dma_start(b1t[:], b1_d.ap().rearrange("(f p) -> p f", p=128))
            nc.sync.dma_start(b2t[:], b2_d.ap().rearrange("(m p) -> p m", p=128))
            v.memset(ones_col[:], 1.0)
            v.memset(ones_row[:], 1.0)
            v.memset(onesq[:], 1.0)
            gp.affine_select(onesq[:], onesq[:], [[1, 128]], ALU.is_equal, 0.0,
                             base=0, channel_multiplier=-1)
            v.tensor_copy(identR[:], onesq[:])
            v.tensor_copy(ident5[:].rearrange("p (j c) -> p j c", j=M),
                          onesq[:].rearrange("p (j c) -> p j c", j=1)
                               .broadcast_to([128, M, 128]))

            # -------- state tiles --------
            fh = [cp.tile([128, KD * RPC], WT, name=f"fh{j}") for j in range(M)]
            gh = [cp.tile([128, KD * RPC], BF, name=f"gh{j}") for j in range(M)]
            z0 = cp.tile([128, KD * RPC], WT)
            z1 = cp.tile([128, KD * RPC], WT)
            z320 = cp.tile([128, KD * RPC], FP)
            z321 = cp.tile([128, KD * RPC], FP)
            hfull = cp.tile([128, KF * RPC], WT)
            junkV = cp.tile([128, RPC], BF)
            junkA = cp.tile([128, RPC], BF)
            dAm = cp.tile([128, 4], FP)               # <g,g> partials by m
            dVm = cp.tile([128, 16], FP)              # (j,m) partials, j-major
            redp = cp.tile([1, 24], FP)
            red4 = cp.tile([1, 196], FP)
            t98 = cp.tile([1, 98], FP)
            red2 = cp.tile([1, 49], FP)
            r5 = cp.tile([1, 5], FP)
            Pg = [cp.tile([1, 25], FP, name=f"pg{j}") for j in range(2)]
            Au = cp.tile([1, 20], FP)      # augmented [HTH | HTy] as [1,4,5]
            u4 = cp.tile([1, 4], FP)
            st4 = cp.tile([1, 4], FP)
            rcp = cp.tile([1, 1], FP)
            rowp = cp.tile([1, 5], FP)
            t45 = cp.tile([1, 20], FP)
            csum = cp.tile([1, 1], FP)
            coeffs = cp.tile([1, 5], FP)
            dumout = cp.tile([1, 4], FP)
            pacev = cp.tile([1, 4], WT)
            chain = [cp.tile([1, 64], WT, name=f"ch{j}") for j in range(NCH + 1)]

            v.memset(dAm[:], 0.0)
            v.memset(dVm[:], 0.0)
            v.memset(redp[:], 0.0)
            v.memset(Pg[0][:], 0.0)
            v.memset(Pg[1][:], 0.0)

            # warm up the collective path (first AllReduce after load pays a
            # large one-time latency).
            n_warm = int(_os.environ.get("K_CC_WARMUP", "2"))

            def warm_ar():
                wcc_in = dp.tile([1, 49], FP, tag="cci", name="wcci")
                wcc_ag = dp.tile([4, 49], FP, tag="cco", name="wccag")
                gp.dma_start(wcc_in[0:1, 0:24], redp[:])
                gp.collective_compute(
                    "AllGather", ALU.bypass, replica_groups=RGROUPS,
                    ins=[wcc_in.opt()], outs=[wcc_ag.opt()],
                )

            for w in range(n_warm):
                warm_ar()

            # ---------------- prolog: xwx = Wx.T @ xT + b1 ----------------
            with tc.tile_pool(name="prolog", bufs=1) as pro:
                xTs = pro.tile([128, KD * RPC], WT)
                Wxp = pro.tile([128, KD * F], WT)
                # two hardware DMA queues in parallel: Wx on the scalar
                # engine's queue, everything else on sync
                for k in range(KD):
                    sc.dma_start(Wxp[:, k * F:(k + 1) * F],
                                 Wx_d[k * 128:(k + 1) * 128, :])
                for k in range(KD):
                    nc.sync.dma_start(xTs[:, k * RPC:(k + 1) * RPC],
                                      xT_d[k * 128:(k + 1) * 128, :])
                for f in range(KF):
                    nc.sync.dma_start(W2p[:, f * D:(f + 1) * D],
                                      W2_d[f * 128:(f + 1) * 128, :])
                for k in range(KD):
                    nc.sync.dma_start(W1p[:, k * F:(k + 1) * F],
                                      W1_d[k * 128:(k + 1) * 128, :])
                for f in range(KF):
                    ps1 = pp1.tile([128, RPC], FP, tag="ps1", name="ps1")
                    for k in range(KD):
                        nc.tensor.matmul(
                            ps1[:],
                            Wxp[:, k * F + f * 128: k * F + (f + 1) * 128],
                            xTs[:, k * RPC:(k + 1) * RPC],
                            start=(k == 0), stop=(k == KD - 1),
                        )
                    sc.activation(xwxp[:, f * RPC:(f + 1) * RPC], ps1[:],
                                  ACT.Identity, bias=b1t[:, f:f + 1], scale=1.0)

            # ---------------- main loop (fully unrolled) ----------------
            z_mm = None   # bf16 AP of z_i for matmuls (None for i=0 -> zeros)
            z_sub = None  # AP used by the g subtraction (fp32 from i=6 on)
            for i in range(MAX_ITER):
                slot = i % M
                last = (i == MAX_ITER - 1)
                f_t, g_t = fh[slot], gh[slot]

                # ---- h phase: hfull = tanh(z @ W1 + xwx) ----
                for f in range(KF):
                    fs = slice(f * RPC, (f + 1) * RPC)
                    if i == 0:
                        sc.activation(hfull[:, fs], xwxp[:, fs], ACT.Tanh)
                        continue
                    ps1 = pp1.tile([128, RPC], FP, tag="ps1", name="ps1")
                    for k in range(KD):
                        nc.tensor.matmul(
                            ps1[:],
                            W1p[:, k * F + f * 128: k * F + (f + 1) * 128],
                            z_mm[:, k * RPC:(k + 1) * RPC],
                            start=(k == 0), stop=(k == KD - 1),
                        )
                    # xwx folded in on the DVE (PSUM read), tanh from SBUF
                    hb = hp.tile([128, RPC], BF, tag="hb", name="hb")
                    v.scalar_tensor_tensor(hb[:], ps1[:], 1.0, xwxp[:, fs],
                                           op0=ALU.bypass, op1=ALU.add)
                    sc.activation(hfull[:, fs], hb[:], ACT.Tanh)

                # ---- W2 phase, m-outer; f/g/dot partials trail each m ----
                njd = min(i, M - 1)
                for m in range(MD):
                    ms = slice(m * RPC, (m + 1) * RPC)
                    ps2 = pp2.tile([128, RPC], FP, tag="ps2", name="ps2")
                    for f in range(KF):
                        nc.tensor.matmul(
                            ps2[:],
                            W2p[:, f * D + m * 128: f * D + (m + 1) * 128],
                            hfull[:, f * RPC:(f + 1) * RPC],
                            start=(f == 0), stop=(f == KF - 1),
                        )
                    sc.activation(f_t[:, ms], ps2[:],
                                  ACT.Identity, bias=b2t[:, m:m + 1], scale=1.0)
                    if i == 0:
                        v.tensor_scalar(g_t[:, ms], ps2[:], b2t[:, m:m + 1],
                                        None, op0=ALU.add)
                    else:
                        v.scalar_tensor_tensor(g_t[:, ms], ps2[:],
                                               b2t[:, m:m + 1], z_sub[:, ms],
                                               op0=ALU.add, op1=ALU.subtract)
                    sc.activation(junkA[:], g_t[:, ms], ACT.Square,
                                  accum_out=dAm[:, m:m + 1])
                    for j in range(1, njd + 1):
                        v.scalar_tensor_tensor(
                            junkV[:], g_t[:, ms], 1.0, gh[(i - j) % M][:, ms],
                            op0=ALU.bypass, op1=ALU.mult,
                            accum_out=dVm[:, (j - 1) * 4 + m:(j - 1) * 4 + m + 1])

                # ---- partition-reduce dot partials ----
                pball = pps.tile([128, 32], FP, tag="psmall", name="pball")
                psd = pball[0:1, 0:20]
                nc.tensor.matmul(psd[:, 0:4], ones_col[:], dAm[:],
                                 start=True, stop=True)
                nc.tensor.matmul(psd[:, 4:20], ones_col[:], dVm[:],
                                 start=True, stop=True)
                do_ar = i >= M
                if do_ar:
                    cc_in = dp.tile([1, 49], FP, tag="cci", name="cci")
                    cc_ag = dp.tile([4, 49], FP, tag="cco", name="ccag")
                    sc.activation(redp[:, 0:20], psd, ACT.Copy)
                    nc.sync.dma_start(cc_in[0:1, 0:20], redp[:, 0:20])
                    if i == M:
                        # fuse the early-phase Gram reduction into the same
                        # collective: ship the local P alongside the dots
                        nc.sync.dma_start(cc_in[0:1, 24:49], Pg[(i + 1) % 2][:])
                    gp.collective_compute(
                        "AllGather", ALU.bypass, replica_groups=RGROUPS,
                        ins=[cc_in.opt()], outs=[cc_ag.opt()],
                    )

                # HAM keep-warm: one long accumulation group of junk matmuls
                # (closed after the solve and READ once, so DCE keeps them)
                # fills the AllReduce wait; solve-keyed members tick the PE
                # through the Gauss-Jordan.
                pdum = None
                if do_ar and (NDUM > 0 or NCH > 0):
                    pdum = pps.tile([1, 512], FP, tag="dum", name="pdum")
                    for k in range(NDUM):
                        nc.tensor.matmul(pdum[:], identR[0:1, 0:1],
                                         xwxp[0:1, 0:512],
                                         start=(k == 0), stop=False)
                    # DMA-chain-paced ticks: each link lands ~1.3us after the
                    # previous, giving the PE a heartbeat through the
                    # collective wait at negligible power.
                    for k in range(NCH):
                        if k == 0:
                            # anchor the chain to this iteration's tail: g_t
                            # is finished exactly when the dots ship out
                            sc.dma_start(chain[1][:], g_t[0:1, 0:64])
                        else:
                            sc.dma_start(chain[k + 1][:], chain[k][:])
                        nc.tensor.matmul(pdum[0:1, 0:64], identR[0:1, 0:1],
                                         chain[k + 1][:],
                                         start=(NDUM == 0 and k == 0),
                                         stop=False)

                if do_ar:
                    nc.sync.dma_start(red4[:],
                                      cc_ag[:].rearrange("a b -> (a b)"))
                    # sum the 4 ranks' partials, then the 4 m-partials
                    v.tensor_tensor(t98[:], red4[:, 0:98], red4[:, 98:196],
                                    op=ALU.add)
                    v.tensor_tensor(red2[:], t98[:, 0:49], t98[:, 49:98],
                                    op=ALU.add)
                    v.tensor_reduce(r5[:],
                                    red2[:, 0:20].rearrange(
                                        "p (j m) -> p j m", j=5),
                                    axis=mybir.AxisListType.X, op=ALU.add)
                else:
                    v.tensor_reduce(r5[:],
                                    psd.rearrange("p (j m) -> p j m", j=5),
                                    axis=mybir.AxisListType.X, op=ALU.add)
                    if i in (2, 4):
                        warm_ar()  # keep the collective path warm

                Pc, Pp = Pg[i % 2], Pg[(i + 1) % 2]
                P3c = Pc[:].rearrange("p (a b) -> p a b", a=5)
                P3p = Pp[:].rearrange("p (a b) -> p a b", a=5)
                if i == M:
                    # previous P arrives globally-reduced in the payload
                    v.tensor_copy(Pp[:], red2[:, 24:49])

                if i < M:
                    # ---- P shift + insert (r5: [<g,g>, j1..j4]) ----
                    v.tensor_copy(P3c[:, 1:5, 1:5], P3p[:, 0:4, 0:4])
                    v.tensor_copy(Pc[:, 0:5], r5[:, 0:5])
                    v.tensor_copy(Pc[:, 5:25:5], r5[:, 1:5])
                    z_mm = f_t[:]
                    z_sub = f_t[:]
                    continue

                # ---- augmented [HTH + lam I | HTy] straight from r5 and the
                # OLD P (the shifted-P copy happens off the critical path) ----
                A3 = Au[:].rearrange("p (a b) -> p a b", a=4)
                # t[a,b] = r5[a] - P_old[a-1,b-1]
                v.tensor_tensor(A3[:, :, 0:4],
                                r5[:, 1:5].rearrange("p (a b) -> p a b", b=1)
                                          .broadcast_to([1, 4, 4]),
                                P3p[:, 0:4, 0:4], op=ALU.subtract)
                # u4[b] = r5[0] - r5[b]  (equals HTy as well)
                v.scalar_tensor_tensor(u4[:], r5[:, 1:5], -1.0,
                                       r5[:, 0:1].broadcast_to([1, 4]),
                                       op0=ALU.mult, op1=ALU.add)
                v.tensor_tensor(A3[:, :, 0:4],
                                u4[:].rearrange("p (a b) -> p a b", a=1)
                                     .broadcast_to([1, 4, 4]),
                                A3[:, :, 0:4], op=ALU.subtract)
                v.tensor_scalar(st4[:], Au[:, 0:19:6], LAM, None, op0=ALU.add)
                v.tensor_copy(Au[:, 0:19:6], st4[:])
                v.tensor_copy(A3[:, :, 4:5],
                              u4[:].rearrange("p (a b) -> p a b", b=1))
                # P shift + insert for the next iteration (not on the path)
                v.tensor_copy(P3c[:, 1:5, 1:5], P3p[:, 0:4, 0:4])
                v.tensor_copy(Pc[:, 0:5], r5[:, 0:5])
                v.tensor_copy(Pc[:, 5:25:5], r5[:, 1:5])

                # ---- Gauss-Jordan (no pivoting; HTH is SPD + lam I) ----
                # a dummy PE matmul after each pivot keeps the clock hot
                for p in range(4):
                    v.reciprocal(rcp[:], Au[:, p * 6:p * 6 + 1])
                    v.tensor_scalar(rowp[:], Au[:, p * 5:(p + 1) * 5], rcp[:],
                                    None, op0=ALU.mult)
                    v.tensor_tensor(t45[:].rearrange("p (a b) -> p a b", a=4),
                                    A3[:, :, p:p + 1].broadcast_to([1, 4, 5]),
                                    rowp[:].rearrange("p (a b) -> p a b", a=1)
                                           .broadcast_to([1, 4, 5]),
                                    op=ALU.mult)
                    v.tensor_tensor(A3, A3,
                                    t45[:].rearrange("p (a b) -> p a b", a=4),
                                    op=ALU.subtract)
                    v.tensor_copy(Au[:, p * 5:(p + 1) * 5], rowp[:])
                    if pdum is not None:
                        # pace the PE through the solve with all-f32r members
                        v.tensor_copy(pacev[:], rowp[:, 0:4])
                        nc.tensor.matmul(pdum[0:1, 0:4], identR[0:1, 0:1],
                                         pacev[:], start=False, stop=False)

                # gamma = Au[:, 4:20:5]; coeffs = [1 - sum(gamma), gamma]
                v.tensor_reduce(csum[:], Au[:, 4:20:5],
                                axis=mybir.AxisListType.X, op=ALU.add)
                v.tensor_scalar(coeffs[:, 0:1], csum[:], -1.0, 1.0,
                                op0=ALU.mult, op1=ALU.add)
                v.tensor_copy(coeffs[:, 1:5], Au[:, 4:20:5])

                # broadcast coeffs to all partitions, build scaled identities
                psb = pball[:, 20:25]
                nc.tensor.matmul(psb, ones_row[:], coeffs[:],
                                 start=True, stop=True)
                v.tensor_tensor(
                    identAll[:].rearrange("p (j c) -> p j c", j=M),
                    ident5[:].rearrange("p (j c) -> p j c", j=M),
                    psb[:].rearrange("p (j c) -> p j c", c=1)
                          .broadcast_to([128, M, 128]),
                    op=ALU.mult)

                # close + read the keep-warm group so it survives DCE
                if pdum is not None:
                    v.tensor_copy(pacev[:], coeffs[:, 0:4])
                    nc.tensor.matmul(pdum[0:1, 0:4], identR[0:1, 0:1],
                                     pacev[:], start=False, stop=True)
                    sc.activation(dumout[:], pdum[0:1, 0:4], ACT.Copy)

                # ---- z_{i+1} = sum_k c_k f_{i-k} on the PE ----
                zn = z0 if (i % 2 == 0) else z1
                zn32 = z320 if (i % 2 == 0) else z321
                for kc in range(KD):
                    ks = slice(kc * RPC, (kc + 1) * RPC)
                    psz = ppz.tile([128, RPC], FP, tag="psz", name="psz")
                    for j in range(M):
                        nc.tensor.matmul(psz[:],
                                         identAll[:, j * 128:(j + 1) * 128],
                                         fh[(i - j) % M][:, ks],
                                         start=(j == 0), stop=(j == M - 1))
                    sc.activation(zn[:, ks], psz[:], ACT.Identity)
                    v.tensor_copy(zn32[:, ks], psz[:])
                z_mm = zn[:]
                z_sub = zn32[:]

            for k in range(KD):
                nc.sync.dma_start(zout_d[k * 128:(k + 1) * 128, :],
                                  z_sub[:, k * RPC:(k + 1) * RPC])

    nc.compile()
    nc.finalize()
    return nc


_NC = None


def _get_nc():
    global _NC
    if _NC is None:
        nc = bacc.Bacc(trn_type="TRN2", debug=False, num_devices=NCORES)
        _NC = _emit(nc)
    return _NC


def kernel(**inputs):
    import ml_dtypes
    bf = ml_dtypes.bfloat16
    x = np.ascontiguousarray(np.asarray(inputs["x_input"], dtype=np.float32))
    W1 = np.ascontiguousarray(np.asarray(inputs["W1"], dtype=np.float32).astype(bf))
    Wx = np.ascontiguousarray(np.asarray(inputs["Wx"], dtype=np.float32).astype(bf))
    b1 = np.ascontiguousarray(np.asarray(inputs["b1"], dtype=np.float32))
    W2 = np.ascontiguousarray(np.asarray(inputs["W2"], dtype=np.float32).astype(bf))
    b2 = np.ascontiguousarray(np.asarray(inputs["b2"], dtype=np.float32))

    nc = _get_nc()
    in_maps = []
    for c in range(NCORES):
        b, s0 = c // 4, (c % 4) * RPC
        in_maps.append({
            "xT": np.ascontiguousarray(x[b, s0:s0 + RPC, :].T.astype(bf)),
            "W1": W1, "Wx": Wx, "W2": W2, "b1": b1, "b2": b2,
        })
    res = run_bass_kernel_spmd(nc, in_maps, core_ids=list(range(NCORES)))
    out = np.zeros((B, S, D), np.float32)
    for c, om in enumerate(res.results):
        b, s0 = c // 4, (c % 4) * RPC
        out[b, s0:s0 + RPC, :] = om["zT_out"].T
    return out


# revision 49
# speedup vs baseline: 1.0510x; 1.0077x over previous
"""Trainium2 Bass kernel for the DeepEquilibriumModel (Anderson-accelerated DEQ).

Problem: 12 unrolled iterations of
    f(z) = tanh(z @ W1 + x @ Wx + b1) @ W2 + b2
with Anderson mixing (M=5, beta=1, lam=1e-4) from iteration 5 on.

Sharding: pure data parallelism over the 2048 = B*S rows; 8 cores get 256
rows each (cores 0-3 hold batch 0, cores 4-7 batch 1). Weights replicated.
The Anderson normal equations need global row sums per batch element, done
with a tiny per-group AllReduce (groups {0..3} / {4..7}).

v3 structure (tuned to measured engine rates):
  * h-loop: per f-chunk, ps1 = identity@xwx + sum_k W1[k,f]@z[k]; tanh to a
    full h tile.  W2 phase is m-outer (one PSUM bank at a time) so f, g and
    the Gram dot partials trail each m-chunk instead of waiting for all.
  * dots are m-chunked (16 small DVE ops + 4 ACT squares) -> only ~2.5us of
    tail after the last W2 matmul before the AllReduce can launch.
  * Early iterations (0..3) do NO AllReduce: the Gram matrix P is built from
    local partials and reduced ONCE at i=4 (P is linear in the dots), which
    avoids backing up the collective stream.
  * z_{i+1} = sum_k c_k f_{i-k} runs on the PE as 5 scaled-identity matmuls
    per k-chunk (identC_k built by DVE from the broadcast coefficients),
    with ACT copying PSUM->SBUF; ~3us instead of ~8us of DVE stt chain.
  * 4x4 solve via vectorized Gauss-Jordan (SPD + lam*I, no pivoting) on
    [1,4,5] views; builds are fused (~26 tiny DVE ops total).
  * HAM keep-warm: free-running dummy matmuls fill the AllReduce wait and
    solve-keyed dummies tick the PE through the Gauss-Jordan so the clock
    gate never drops the PE to 1.2 GHz mid-iteration.
"""

import numpy as np

from concourse import bacc, bass, mybir, tile
from concourse.bass_utils import run_bass_kernel_spmd

import os as _os

B, S, D, F = 2, 1024, 512, 2048
MAX_ITER, M, LAM = int(_os.environ.get("K_ITERS", "12")), 5, 1e-4
NCORES = 8
RPC = (B * S) // NCORES      # rows per core = 256
KD = D // 128                # 4 k-chunks over D
KF = F // 128                # 16 k-chunks over F
MD = D // 128                # 4 output chunks over D
NDUM = int(_os.environ.get("K_NDUM", "0"))
NCH = int(_os.environ.get("K_NCH", "0"))

FP = mybir.dt.float32
FPR = mybir.dt.float32r
BF = mybir.dt.bfloat16
ALU = mybir.AluOpType
ACT = mybir.ActivationFunctionType

RGROUPS = [[0, 1, 2, 3], [4, 5, 6, 7]]
WT = BF


def _f32(ap):
    return ap.bitcast(FP)


def _emit(nc: bass.Bass):
    v = nc.vector
    sc = nc.scalar
    gp = nc.gpsimd

    # ---------------- DRAM I/O ----------------
    xT_d = nc.dram_tensor("xT", [D, RPC], WT, kind="ExternalInput")
    W1_d = nc.dram_tensor("W1", [D, F], WT, kind="ExternalInput")
    Wx_d = nc.dram_tensor("Wx", [D, F], WT, kind="ExternalInput")
    W2_d = nc.dram_tensor("W2", [F, D], WT, kind="ExternalInput")
    b1_d = nc.dram_tensor("b1", [F], FP, kind="ExternalInput")
    b2_d = nc.dram_tensor("b2", [D], FP, kind="ExternalInput")
    zout_d = nc.dram_tensor("zT_out", [D, RPC], FP, kind="ExternalOutput")

    with tile.TileContext(nc) as tc:
        with (
            tc.tile_pool(name="const", bufs=1) as cp,
            tc.tile_pool(name="hbp", bufs=3) as hp,
            tc.tile_pool(name="ps1p", bufs=2, space="PSUM") as pp1,
            tc.tile_pool(name="ps2p", bufs=2, space="PSUM") as pp2,
            tc.tile_pool(name="pszp", bufs=2, space="PSUM") as ppz,
            tc.tile_pool(name="pssm", bufs=1, space="PSUM") as pps,
            tc.tile_pool(name="dram", bufs=2, space="DRAM") as dp,
        ):
            # ---------------- constants / weights ----------------
            W1p = cp.tile([128, KD * F], WT)          # (k,f) at [:, k*F + f*128]
            W2p = cp.tile([128, KF * D], WT)          # (f,m) at [:, f*D + m*128]
            xwxp = cp.tile([128, KF * RPC], WT)       # f at [:, f*RPC]
            b1t = cp.tile([128, KF], FP)
            b2t = cp.tile([128, MD], FP)
            ones_col = cp.tile([128, 1], FP)
            ones_row = cp.tile([1, 128], FP)
            onesq = cp.tile([128, 128], FP)
            identR = cp.tile([128, 128], WT)
            ident5 = cp.tile([128, M * 128], WT)
            identAll = cp.tile([128, M * 128], WT)

            nc.sync.dma_start(b1t[:], b1_d.ap().rearrange("(f p) -> p f", p=128))
            nc.sync.dma_start(b2t[:], b2_d.ap().rearrange("(m p) -> p m", p=128))
            v.memset(ones_col[:], 1.0)
            v.memset(ones_row[:], 1.0)
            v.memset(onesq[:], 1.0)
            gp.affine_select(onesq[:], onesq[:], [[1, 128]], ALU.is_equal, 0.0,
                             base=0, channel_multiplier=-1)
            v.tensor_copy(identR[:], onesq[:])
            v.tensor_copy(ident5[:].rearrange("p (j c) -> p j c", j=M),
                          onesq[:].rearrange("p (j c) -> p j c", j=1)
                               .broadcast_to([128, M, 128]))

            # -------- state tiles --------
            fh = [cp.tile([128, KD * RPC], WT, name=f"fh{j}") for j in range(M)]
            gh = [cp.tile([128, KD * RPC], BF, name=f"gh{j}") for j in range(M)]
            z0 = cp.tile([128, KD * RPC], WT)
            z1 = cp.tile([128, KD * RPC], WT)
            z320 = cp.tile([128, KD * RPC], FP)
            z321 = cp.tile([128, KD * RPC], FP)
            hfull = cp.tile([128, KF * RPC], WT)
            junkV = cp.tile([128, RPC], BF)
            junkA = cp.tile([128, RPC], BF)
            dAm = cp.tile([128, 4], FP)               # <g,g> partials by m
            dVm = cp.tile([128, 16], FP)              # (j,m) partials, j-major
            redp = cp.tile([1, 24], FP)
            red4 = cp.tile([1, 196], FP)
            t98 = cp.tile([1, 98], FP)
            red2 = cp.tile([1, 49], FP)
            r5 = cp.tile([1, 5], FP)
            Pg = [cp.tile([1, 25], FP, name=f"pg{j}") for j in range(2)]
            Au = cp.tile([1, 20], FP)      # augmented [HTH | HTy] as [1,4,5]
            u4 = cp.tile([1, 4], FP)
            st4 = cp.tile([1, 4], FP)
            rcp = cp.tile([1, 1], FP)
            rowp = cp.tile([1, 5], FP)
            t45 = cp.tile([1, 20], FP)
            csum = cp.tile([1, 1], FP)
            coeffs = cp.tile([1, 5], FP)
            dumout = cp.tile([1, 4], FP)
            pacev = cp.tile([1, 4], WT)
            chain = [cp.tile([1, 64], WT, name=f"ch{j}") for j in range(NCH + 1)]

            v.memset(dAm[:], 0.0)
            v.memset(dVm[:], 0.0)
            v.memset(redp[:], 0.0)
            v.memset(Pg[0][:], 0.0)
            v.memset(Pg[1][:], 0.0)

            # warm up the collective path (first AllReduce after load pays a
            # large one-time latency).
            n_warm = int(_os.environ.get("K_CC_WARMUP", "2"))

            def warm_ar():
                wcc_in = dp.tile([1, 49], FP, tag="cci", name="wcci")
                wcc_ag = dp.tile([4, 49], FP, tag="cco", name="wccag")
                gp.dma_start(wcc_in[0:1, 0:24], redp[:])
                gp.collective_compute(
                    "AllGather", ALU.bypass, replica_groups=RGROUPS,
                    ins=[wcc_in.opt()], outs=[wcc_ag.opt()],
                )

            for w in range(n_warm):
                warm_ar()

            # ---------------- prolog: xwx = Wx.T @ xT + b1 ----------------
            with tc.tile_pool(name="prolog", bufs=1) as pro:
                xTs = pro.tile([128, KD * RPC], WT)
                Wxp = pro.tile([128, KD * F], WT)
                # two hardware DMA queues in parallel: Wx on the scalar
                # engine's queue, everything else on sync
                for k in range(KD):
                    sc.dma_start(Wxp[:, k * F:(k + 1) * F],
                                 Wx_d[k * 128:(k + 1) * 128, :])
                for k in range(KD):
                    nc.sync.dma_start(xTs[:, k * RPC:(k + 1) * RPC],
                                      xT_d[k * 128:(k + 1) * 128, :])
                for f in range(KF):
                    nc.sync.dma_start(W2p[:, f * D:(f + 1) * D],
                                      W2_d[f * 128:(f + 1) * 128, :])
                for k in range(KD):
                    nc.sync.dma_start(W1p[:, k * F:(k + 1) * F],
                                      W1_d[k * 128:(k + 1) * 128, :])
                for f in range(KF):
                    ps1 = pp1.tile([128, RPC], FP, tag="ps1", name="ps1")
                    for k in range(KD):
                        nc.tensor.matmul(
                            ps1[:],
                            Wxp[:, k * F + f * 128: k * F + (f + 1) * 128],
                            xTs[:, k * RPC:(k + 1) * RPC],
                            start=(k == 0), stop=(k == KD - 1),
                        )
                    sc.activation(xwxp[:, f * RPC:(f + 1) * RPC], ps1[:],
                                  ACT.Identity, bias=b1t[:, f:f + 1], scale=1.0)

            # ---------------- main loop (fully unrolled) ----------------
            z_mm = None   # bf16 AP of z_i for matmuls (None for i=0 -> zeros)
            z_sub = None  # AP used by the g subtraction (fp32 from i=6 on)
            for i in range(MAX_ITER):
                slot = i % M
                last = (i == MAX_ITER - 1)
                f_t, g_t = fh[slot], gh[slot]

                # ---- h phase: hfull = tanh(z @ W1 + xwx) ----
                for f in range(KF):
                    fs = slice(f * RPC, (f + 1) * RPC)
                    if i == 0:
                        sc.activation(hfull[:, fs], xwxp[:, fs], ACT.Tanh)
                        continue
                    ps1 = pp1.tile([128, RPC], FP, tag="ps1", name="ps1")
                    for k in range(KD):
                        nc.tensor.matmul(
                            ps1[:],
                            W1p[:, k * F + f * 128: k * F + (f + 1) * 128],
                            z_mm[:, k * RPC:(k + 1) * RPC],
                            start=(k == 0), stop=(k == KD - 1),
                        )
                    # xwx folded in on the DVE (PSUM read), tanh from SBUF
                    hb = hp.tile([128, RPC], BF, tag="hb", name="hb")
                    v.scalar_tensor_tensor(hb[:], ps1[:], 1.0, xwxp[:, fs],
                                           op0=ALU.bypass, op1=ALU.add)
                    sc.activation(hfull[:, fs], hb[:], ACT.Tanh)

                # ---- W2 phase, m-outer; f/g/dot partials trail each m ----
                njd = min(i, M - 1)
                for m in range(MD):
                    ms = slice(m * RPC, (m + 1) * RPC)
                    ps2 = pp2.tile([128, RPC], FP, tag="ps2", name="ps2")
                    for f in range(KF):
                        nc.tensor.matmul(
                            ps2[:],
                            W2p[:, f * D + m * 128: f * D + (m + 1) * 128],
                            hfull[:, f * RPC:(f + 1) * RPC],
                            start=(f == 0), stop=(f == KF - 1),
                        )
                    sc.activation(f_t[:, ms], ps2[:],
                                  ACT.Identity, bias=b2t[:, m:m + 1], scale=1.0)
                    if i == 0:
                        v.tensor_scalar(g_t[:, ms], ps2[:], b2t[:, m:m + 1],
                                        None, op0=ALU.add)
                    else:
                        v.scalar_tensor_tensor(g_t[:, ms], ps2[:],
                                               b2t[:, m:m + 1], z_sub[:, ms],
                                               op0=ALU.add, op1=ALU.subtract)
                    sc.activation(junkA[:], g_t[:, ms], ACT.Square,
                                  accum_out=dAm[:, m:m + 1])
                    for j in range(1, njd + 1):
                        v.scalar_tensor_tensor(
                            junkV[:], g_t[:, ms], 1.0, gh[(i - j) % M][:, ms],
                            op0=ALU.bypass, op1=ALU.mult,
                            accum_out=dVm[:, (j - 1) * 4 + m:(j - 1) * 4 + m + 1])

                # ---- partition-reduce dot partials ----
                pball = pps.tile([128, 32], FP, tag="psmall", name="pball")
                psd = pball[0:1, 0:20]
                nc.tensor.matmul(psd[:, 0:4], ones_col[:], dAm[:],
                                 start=True, stop=True)
                nc.tensor.matmul(psd[:, 4:20], ones_col[:], dVm[:],
                                 start=True, stop=True)
                do_ar = i >= M
                if do_ar:
                    cc_in = dp.tile([1, 49], FP, tag="cci", name="cci")
                    cc_ag = dp.tile([4, 49], FP, tag="cco", name="ccag")
                    sc.activation(redp[:, 0:20], psd, ACT.Copy)
                    nc.sync.dma_start(cc_in[0:1, 0:20], redp[:, 0:20])
                    if i == M:
                        # fuse the early-phase Gram reduction into the same
                        # collective: ship the local P alongside the dots
                        nc.sync.dma_start(cc_in[0:1, 24:49], Pg[(i + 1) % 2][:])
                    gp.collective_compute(
                        "AllGather", ALU.bypass, replica_groups=RGROUPS,
                        ins=[cc_in.opt()], outs=[cc_ag.opt()],
                    )

                # HAM keep-warm: one long accumulation group of junk matmuls
                # (closed after the solve and READ once, so DCE keeps them)
                # fills the AllReduce wait; solve-keyed members tick the PE
                # through the Gauss-Jordan.
                pdum = None
                if do_ar and (NDUM > 0 or NCH > 0):
                    pdum = pps.tile([1, 512], FP, tag="dum", name="pdum")
                    for k in range(NDUM):
                        nc.tensor.matmul(pdum[:], identR[0:1, 0:1],
                                         xwxp[0:1, 0:512],
                                         start=(k == 0), stop=False)
                    # DMA-chain-paced ticks: each link lands ~1.3us after the
                    # previous, giving the PE a heartbeat through the
                    # collective wait at negligible power.
                    for k in range(NCH):
                        if k == 0:
                            # anchor the chain to this iteration's tail: g_t
                            # is finished exactly when the dots ship out
                            sc.dma_start(chain[1][:], g_t[0:1, 0:64])
                        else:
                            sc.dma_start(chain[k + 1][:], chain[k][:])
                        nc.tensor.matmul(pdum[0:1, 0:64], identR[0:1, 0:1],
                                         chain[k + 1][:],
                                         start=(NDUM == 0 and k == 0),
                                         stop=False)

                if do_ar:
                    nc.sync.dma_start(red4[:],
                                      cc_ag[:].rearrange("a b -> (a b)"))
                    # sum the 4 ranks' partials, then the 4 m-partials
                    v.tensor_tensor(t98[:], red4[:, 0:98], red4[:, 98:196],
                                    op=ALU.add)
                    v.tensor_tensor(red2[:], t98[:, 0:49], t98[:, 49:98],
                                    op=ALU.add)
                    v.tensor_reduce(r5[:],
                                    red2[:, 0:20].rearrange(
                                        "p (j m) -> p j m", j=5),
                                    axis=mybir.AxisListType.X, op=ALU.add)
                else:
                    v.tensor_reduce(r5[:],
                                    psd.rearrange("p (j m) -> p j m", j=5),
                                    axis=mybir.AxisListType.X, op=ALU.add)
                    if i in (2, 3, 4):
                        # warm the collective path; the input DMA reads this
                        # iteration's g so the warm gather fires HERE, not
                        # hoisted to the top of the program
                        wcc_in = dp.tile([1, 49], FP, tag="cci", name="wcci")
                        wcc_ag = dp.tile([4, 49], FP, tag="cco", name="wccag")
                        gp.dma_start(wcc_in[0:1, 0:24], g_t[0:1, 0:24])
                        gp.collective_compute(
                            "AllGather", ALU.bypass, replica_groups=RGROUPS,
                            ins=[wcc_in.opt()], outs=[wcc_ag.opt()],
                        )

                Pc, Pp = Pg[i % 2], Pg[(i + 1) % 2]
                P3c = Pc[:].rearrange("p (a b) -> p a b", a=5)
                P3p = Pp[:].rearrange("p (a b) -> p a b", a=5)
                if i == M:
                    # previous P arrives globally-reduced in the payload
                    v.tensor_copy(Pp[:], red2[:, 24:49])

                if i < M:
                    # ---- P shift + insert (r5: [<g,g>, j1..j4]) ----
                    v.tensor_copy(P3c[:, 1:5, 1:5], P3p[:, 0:4, 0:4])
                    v.tensor_copy(Pc[:, 0:5], r5[:, 0:5])
                    v.tensor_copy(Pc[:, 5:25:5], r5[:, 1:5])
                    z_mm = f_t[:]
                    z_sub = f_t[:]
                    continue

                # ---- augmented [HTH + lam I | HTy] straight from r5 and the
                # OLD P (the shifted-P copy happens off the critical path) ----
                A3 = Au[:].rearrange("p (a b) -> p a b", a=4)
                # t[a,b] = r5[a] - P_old[a-1,b-1]
                v.tensor_tensor(A3[:, :, 0:4],
                                r5[:, 1:5].rearrange("p (a b) -> p a b", b=1)
                                          .broadcast_to([1, 4, 4]),
                                P3p[:, 0:4, 0:4], op=ALU.subtract)
                # u4[b] = r5[0] - r5[b]  (equals HTy as well)
                v.scalar_tensor_tensor(u4[:], r5[:, 1:5], -1.0,
                                       r5[:, 0:1].broadcast_to([1, 4]),
                                       op0=ALU.mult, op1=ALU.add)
                v.tensor_tensor(A3[:, :, 0:4],
                                u4[:].rearrange("p (a b) -> p a b", a=1)
                                     .broadcast_to([1, 4, 4]),
                                A3[:, :, 0:4], op=ALU.subtract)
                v.tensor_scalar(st4[:], Au[:, 0:19:6], LAM, None, op0=ALU.add)
                v.tensor_copy(Au[:, 0:19:6], st4[:])
                v.tensor_copy(A3[:, :, 4:5],
                              u4[:].rearrange("p (a b) -> p a b", b=1))
                # P shift + insert for the next iteration (not on the path)
                v.tensor_copy(P3c[:, 1:5, 1:5], P3p[:, 0:4, 0:4])
                v.tensor_copy(Pc[:, 0:5], r5[:, 0:5])
                v.tensor_copy(Pc[:, 5:25:5], r5[:, 1:5])

                # ---- Gauss-Jordan (no pivoting; HTH is SPD + lam I) ----
                # a dummy PE matmul after each pivot keeps the clock hot
                for p in range(4):
                    v.reciprocal(rcp[:], Au[:, p * 6:p * 6 + 1])
                    v.tensor_scalar(rowp[:], Au[:, p * 5:(p + 1) * 5], rcp[:],
                                    None, op0=ALU.mult)
                    v.tensor_tensor(t45[:].rearrange("p (a b) -> p a b", a=4),
                                    A3[:, :, p:p + 1].broadcast_to([1, 4, 5]),
                                    rowp[:].rearrange("p (a b) -> p a b", a=1)
                                           .broadcast_to([1, 4, 5]),
                                    op=ALU.mult)
                    v.tensor_tensor(A3, A3,
                                    t45[:].rearrange("p (a b) -> p a b", a=4),
                                    op=ALU.subtract)
                    v.tensor_copy(Au[:, p * 5:(p + 1) * 5], rowp[:])
                    if pdum is not None:
                        # pace the PE through the solve with all-f32r members
                        v.tensor_copy(pacev[:], rowp[:, 0:4])
                        nc.tensor.matmul(pdum[0:1, 0:4], identR[0:1, 0:1],
                                         pacev[:], start=False, stop=False)

                # gamma = Au[:, 4:20:5]; coeffs = [1 - sum(gamma), gamma]
                v.tensor_reduce(csum[:], Au[:, 4:20:5],
                                axis=mybir.AxisListType.X, op=ALU.add)
                v.tensor_scalar(coeffs[:, 0:1], csum[:], -1.0, 1.0,
                                op0=ALU.mult, op1=ALU.add)
                v.tensor_copy(coeffs[:, 1:5], Au[:, 4:20:5])

                # broadcast coeffs to all partitions, build scaled identities
                psb = pball[:, 20:25]
                nc.tensor.matmul(psb, ones_row[:], coeffs[:],
                                 start=True, stop=True)
                v.tensor_tensor(
                    identAll[:].rearrange("p (j c) -> p j c", j=M),
                    ident5[:].rearrange("p (j c) -> p j c", j=M),
                    psb[:].rearrange("p (j c) -> p j c", c=1)
                          .broadcast_to([128, M, 128]),
                    op=ALU.mult)

                # close + read the keep-warm group so it survives DCE
                if pdum is not None:
                    v.tensor_copy(pacev[:], coeffs[:, 0:4])
                    nc.tensor.matmul(pdum[0:1, 0:4], identR[0:1, 0:1],
                                     pacev[:], start=False, stop=True)
                    sc.activation(dumout[:], pdum[0:1, 0:4], ACT.Copy)

                # ---- z_{i+1} = sum_k c_k f_{i-k} on the PE ----
                zn = z0 if (i % 2 == 0) else z1
                zn32 = z320 if (i % 2 == 0) else z321
                for kc in range(KD):
                    ks = slice(kc * RPC, (kc + 1) * RPC)
                    psz = ppz.tile([128, RPC], FP, tag="psz", name="psz")
                    for j in range(M):
                        nc.tensor.matmul(psz[:],
                                         identAll[:, j * 128:(j + 1) * 128],
                                         fh[(i - j) % M][:, ks],
                                         start=(j == 0), stop=(j == M - 1))
                    sc.activation(zn[:, ks], psz[:], ACT.Identity)
                    v.tensor_copy(zn32[:, ks], psz[:])
                z_mm = zn[:]
                z_sub = zn32[:]

            for k in range(KD):
                nc.sync.dma_start(zout_d[k * 128:(k + 1) * 128, :],
                                  z_sub[:, k * RPC:(k + 1) * RPC])

    nc.compile()
    nc.finalize()
    return nc


_NC = None


def _get_nc():
    global _NC
    if _NC is None:
        nc = bacc.Bacc(trn_type="TRN2", debug=False, num_devices=NCORES)
        _NC = _emit(nc)
    return _NC


def kernel(**inputs):
    import ml_dtypes
    bf = ml_dtypes.bfloat16
    x = np.ascontiguousarray(np.asarray(inputs["x_input"], dtype=np.float32))
    W1 = np.ascontiguousarray(np.asarray(inputs["W1"], dtype=np.float32).astype(bf))
    Wx = np.ascontiguousarray(np.asarray(inputs["Wx"], dtype=np.float32).astype(bf))
    b1 = np.ascontiguousarray(np.asarray(inputs["b1"], dtype=np.float32))
    W2 = np.ascontiguousarray(np.asarray(inputs["W2"], dtype=np.float32).astype(bf))
    b2 = np.ascontiguousarray(np.asarray(inputs["b2"], dtype=np.float32))

    nc = _get_nc()
    in_maps = []
    for c in range(NCORES):
        b, s0 = c // 4, (c % 4) * RPC
        in_maps.append({
            "xT": np.ascontiguousarray(x[b, s0:s0 + RPC, :].T.astype(bf)),
            "W1": W1, "Wx": Wx, "W2": W2, "b1": b1, "b2": b2,
        })
    res = run_bass_kernel_spmd(nc, in_maps, core_ids=list(range(NCORES)))
    out = np.zeros((B, S, D), np.float32)
    for c, om in enumerate(res.results):
        b, s0 = c // 4, (c % 4) * RPC
        out[b, s0:s0 + RPC, :] = om["zT_out"].T
    return out
